# revision 1
# baseline (speedup 1.0000x reference)
"""LoRA QKV fused projection kernel for 8 TRN2 NeuronCores.

Reference computation (T=8192 tokens, HID=4096, D=6144 out, S=8 slots, R=16):
    y = x @ W.T
    a[t,s,i,r] = sum_h x[t,h] * lora_A[s,i,r,h]         (down-proj, all slots)
    a *= onehot(token_to_slot)[t,s] * scaling[s]         (routing gate)
    d[t, :] = concat_i( sum_{s,r} a[t,s,i,r] * B_i[s,:,r] )   (up-proj)
    out = y + d

Sharding: pure data-parallel over tokens. Core c owns tokens
[c*1024, (c+1)*1024) and computes its full [6144, 1024] output column
block; host assembly is a transpose-concat (no reduction).

Per-core dataflow:
  * x shard resident in SBUF as [128(k-part), 32(k-tile), 1024(tok)] bf16,
    streamed in 4-ktile chunks on the sync DMA queue.
  * Phase A: LoRA down-proj aT[(i,sr), t] = A^T x in fp8 e4m3 with
    MatmulPerfMode.DoubleRow: each instruction contracts a k-tile PAIR
    (256 hidden dims), 2x the bf16 rate. Operands stream pre-quantized
    (x*32, A*1024) in one packed table on the scalar queue; PSUM
    accumulates into 6 tiles (3 targets x 2 token halves). fp8 noise
    only touches the LoRA delta (~10% of output norm).
  * Phase B: routing gate (host-built onehot*scaling/(32*1024), expanded
    over rank) applied on DVE: ag = aT * gate, written as bf16.
  * Phase C: per output row-block mb (48): K accumulation of W[mb] @ x
    into 2 psum tiles (token halves): k-tiles 0..27 in bf16 (1 col/cycle
    = full PE rate), the last 2 k-tile pairs in fp8 DoubleRow (2x rate,
    reusing the pax x8 slices). The bf16 W is pre-scaled by SP=2^15 on
    the host so fp8 (x*32)(W*1024) products accumulate coherently; the
    LoRA up-proj B[mb] @ ag[i] (whose psum scale is already SP) is
    accumulated INTO THE SAME psum (start=False), fusing base + delta
    for free. The psum->sbuf copy descales by 1/SP exactly
    (tensor_scalar_mul), then one DMA out per half.

Precision budget (gate 2e-2): bf16 main ~2.3e-3, fp8 on 4/32 k-tiles
~1.3e-2, fp8 LoRA delta ~3.6e-3 -> measured 1.40e-2 total. bf16 beats
fp32r (same PE rate, no single-bank chain stalls, half the LDWEIGHTS
bytes). W streams on the scalar queue, triple buffered. Engines beyond
PE/DVE are kept idle: GpSimd DMA or bulk Act/DVE casts measurably
downclock the PE ~20%.
"""

import numpy as np
import ml_dtypes

# problem shape (hardcoded per harness contract)
T = 8192
HID = 4096
Q_SIZE = 4096
KV_SIZE = 1024
D = Q_SIZE + 2 * KV_SIZE  # 6144
S = 8
R = 16
NCORES = 8
P = 128

TC = T // NCORES          # 1024 tokens per core
MB = D // P               # 48 output row-blocks of 128
KA = HID // P             # 32 k-tiles
NP8 = 3                   # k-tile PAIRS of the main GEMM done in fp8 DoubleRow
KB = KA - 2 * NP8         # k-tiles of the main GEMM done in bf16
KCH = 4                   # k-tiles per streamed x chunk
NH = TC // 512            # 2 token halves (psum bank = 512 fp32)
SP = np.float32(32768.0)  # psum scale SX*SA: bf16 W is pre-scaled by SP so
                          # fp8 (x*32)(W*1024) products accumulate coherently;
                          # the final psum->sbuf copy descales by 1/SP (exact)

_CACHE = {}


def _build_nc():
    import concourse.mybir as mybir
    import concourse.tile as tile
    from concourse import bacc

    bf16 = mybir.dt.bfloat16
    f32 = mybir.dt.float32
    f8 = mybir.dt.float8e4

    nc = bacc.Bacc(None, target_bir_lowering=False, debug=False)

    # ---- DRAM parameters (per-core shapes)
    PAX = 3 * P + TC  # per-(j,pair) packed row: 3 A targets then x tokens
    x_d = nc.declare_dram_parameter("x_sh", [P, KB, TC], bf16, isOutput=False)
    w_d = nc.declare_dram_parameter("w_t", [MB, P, KB, P], bf16, isOutput=False)
    w8_d = nc.declare_dram_parameter("w8_t", [MB, P, NP8, 2, P], f8, isOutput=False)
    pax_d = nc.declare_dram_parameter("pax", [P, KA // 2, 2, PAX], f8, isOutput=False)
    b_d = nc.declare_dram_parameter("b_t", [P, MB, P], bf16, isOutput=False)
    g_d = nc.declare_dram_parameter("gate", [P, TC], f32, isOutput=False)
    y_d = nc.declare_dram_parameter("y_out", [MB, P, TC], f32, isOutput=True)

    with tile.TileContext(nc) as tc:
        with tc.tile_pool(name="xres", bufs=1) as xres_pool, \
             tc.tile_pool(name="wp", bufs=3) as w_pool, \
             tc.tile_pool(name="ab", bufs=1) as ab_pool, \
             tc.tile_pool(name="agp", bufs=1) as ag_pool, \
             tc.tile_pool(name="stp", bufs=3) as st_pool, \
             tc.tile_pool(name="psum", bufs=8, space="PSUM") as ps_pool:

            # resident operands
            x_res = xres_pool.tile([P, KB, TC], bf16, tag="xres")
            pax_t = ab_pool.tile([P, KA // 2, 2, PAX], f8, tag="pax")
            b_t = ab_pool.tile([P, MB, P], bf16, tag="b")
            gate_t = ab_pool.tile([P, TC], f32, tag="gate")

            # x (bf16, for the main GEMM) streams on the sync queue; the fp8
            # LoRA pack + W stream on the scalar queue in parallel.
            ch0 = 0
            while ch0 < KB:
                cw = min(KCH, KB - ch0)
                nc.sync.dma_start(
                    out=x_res[:, ch0:ch0 + cw, :],
                    in_=x_d[:, ch0:ch0 + cw, :],
                )
                ch0 += cw

            # ---------------- Phase A: LoRA down-proj aT = A @ x ------------
            # fp8 e4m3 DoubleRow: each instruction contracts a k-tile PAIR
            # (256 hidden dims), halving PE time vs bf16. Host pre-scales
            # x by SX and A by SA; 1/(SX*SA) is folded into the gate.
            ps_a = [
                ps_pool.tile([P, 512], f32, tag="ps", name=f"ps_a{i}_{h}")
                for i in range(3) for h in range(NH)
            ]
            JA = KA // 2
            jch = [1, 1, 2, 4, 4, 4]  # j-tiles per streamed chunk
            j0 = 0
            for jc in jch:
                nc.scalar.dma_start(
                    out=pax_t[:, j0:j0 + jc], in_=pax_d[:, j0:j0 + jc])
                for j in range(j0, j0 + jc):
                    for i in range(3):
                        for h in range(NH):
                            nc.tensor.matmul(
                                ps_a[i * NH + h][:],
                                pax_t[:, j, :, i * P:(i + 1) * P],
                                pax_t[:, j, :, 3 * P + h * 512:3 * P + (h + 1) * 512],
                                start=(j == 0), stop=(j == JA - 1),
                                perf_mode=mybir.MatmulPerfMode.DoubleRow,
                            )
                j0 += jc
            nc.sync.dma_start(out=gate_t[:], in_=g_d[:])
            nc.sync.dma_start(out=b_t[:], in_=b_d[:])

            # ---------------- Phase B: routing gate ------------------------
            ag = []
            for i in range(3):
                ag_t = ag_pool.tile([P, TC], bf16, tag=f"ag{i}", name=f"ag{i}")
                for h in range(NH):
                    sl = slice(h * 512, (h + 1) * 512)
                    nc.vector.tensor_mul(ag_t[:, sl], ps_a[i * NH + h][:], gate_t[:, sl])
                ag.append(ag_t)

            # ------------- Phase C: main GEMM + fused LoRA up-proj ----------
            # k-tiles 0..KB-1 in bf16 (W pre-scaled by SP on host); the last
            # NP8 k-tile pairs ride fp8 DoubleRow, reusing the pax x8 slices.
            # Everything accumulates at SP x true scale in one psum group;
            # the copy-out descales by 1/SP exactly.
            for mb in range(MB):
                w_t = w_pool.tile([P, KB, P], bf16, tag="w", name=f"w{mb}")
                nc.scalar.dma_start(out=w_t[:], in_=w_d[mb])
                w8_t = w_pool.tile([P, NP8, 2, P], f8, tag="w8", name=f"w8{mb}")
                nc.scalar.dma_start(out=w8_t[:], in_=w8_d[mb])
                i = 0 if mb < Q_SIZE // P else (1 if mb < (Q_SIZE + KV_SIZE) // P else 2)
                pss = [
                    ps_pool.tile([P, 512], f32, tag="ps", name=f"pm{mb}_{h}")
                    for h in range(NH)
                ]
                for k in range(KB):
                    for h in range(NH):
                        nc.tensor.matmul(
                            pss[h][:],
                            w_t[:, k, :],
                            x_res[:, k, h * 512:(h + 1) * 512],
                            start=(k == 0), stop=False,
                        )
                for jp in range(NP8):
                    j = KB // 2 + jp
                    for h in range(NH):
                        nc.tensor.matmul(
                            pss[h][:],
                            w8_t[:, jp, :, :],
                            pax_t[:, j, :, 3 * P + h * 512:3 * P + (h + 1) * 512],
                            start=False, stop=False,
                            perf_mode=mybir.MatmulPerfMode.DoubleRow,
                        )
                st = st_pool.tile([P, TC], f32, tag="st", name=f"st{mb}")
                for h in range(NH):
                    nc.tensor.matmul(
                        pss[h][:],
                        b_t[:, mb, :],
                        ag[i][:, h * 512:(h + 1) * 512],
                        start=False, stop=True,
                    )
                    nc.vector.tensor_scalar_mul(
                        st[:, h * 512:(h + 1) * 512], pss[h][:],
                        float(1.0 / SP))
                    nc.sync.dma_start(
                        out=y_d[mb, :, h * 512:(h + 1) * 512],
                        in_=st[:, h * 512:(h + 1) * 512],
                    )

    nc.compile()
    return nc


def _get_nc():
    if "nc" not in _CACHE:
        _CACHE["nc"] = _build_nc()
    return _CACHE["nc"]


def _prep_in_maps(x, W, lora_A, lora_B_q, lora_B_k, lora_B_v, scaling, token_to_slot):
    f = np.float32
    bf = ml_dtypes.bfloat16
    x = np.ascontiguousarray(x, dtype=f)
    W = np.ascontiguousarray(W, dtype=f)

    # x shard, moving operand: [c, p, ka, tl]  (h = ka*128 + p, t = c*1024 + tl)
    x_f32 = np.ascontiguousarray(
        x.reshape(NCORES, TC, KA, P).transpose(0, 3, 2, 1))
    x_sh = np.ascontiguousarray(x_f32[:, :, :KB, :]).astype(bf)
    # W stationary: [mb, p, ka, dl]  (d = mb*128 + dl)  -- replicated.
    # bf16 part pre-scaled by SP to match the fp8 psum scale; the last
    # 2*NP8 k-tiles go as fp8(W*1024) DoubleRow pairs.
    w_all = W.reshape(MB, P, KA, P).transpose(0, 3, 2, 1)
    w_t = np.ascontiguousarray(w_all[:, :, :KB, :] * SP).astype(bf)
    # fp8 e4m3 copies for the LoRA down-proj (DoubleRow pairs of k-tiles),
    # packed [A targets | x tokens] per (j, pair) row so each chunk is one
    # DMA; the 1/(SX*SA) descale folds into the gate below.
    SX, SA = np.float32(32.0), np.float32(1024.0)
    f8 = ml_dtypes.float8_e4m3
    w8 = np.ascontiguousarray(
        (w_all[:, :, KB:, :] * SA).astype(f8).reshape(MB, P, NP8, 2, P))
    x8 = (x_f32 * SX).astype(f8).reshape(NCORES, P, KA // 2, 2, TC)
    a_f32 = np.ascontiguousarray(
        np.asarray(lora_A, dtype=f).reshape(S, 3, R, KA, P).transpose(4, 3, 1, 0, 2)
        .reshape(P, KA, 3, S * R))
    a8 = (a_f32 * SA).astype(f8).reshape(P, KA // 2, 2, 3 * S * R)
    pax = np.concatenate(
        [np.broadcast_to(a8, (NCORES,) + a8.shape), x8], axis=-1)
    pax = np.ascontiguousarray(pax)
    # LoRA B stationary: [(s r), mb, dl] -- replicated
    bq = np.asarray(lora_B_q, dtype=f).transpose(0, 2, 1).reshape(S * R, Q_SIZE)
    bk = np.asarray(lora_B_k, dtype=f).transpose(0, 2, 1).reshape(S * R, KV_SIZE)
    bv = np.asarray(lora_B_v, dtype=f).transpose(0, 2, 1).reshape(S * R, KV_SIZE)
    b_t = np.ascontiguousarray(
        np.concatenate([bq, bk, bv], axis=1).reshape(S * R, MB, P)).astype(bf)
    # routing gate, expanded over ranks: [c, (s r), tl]. The LoRA psum is
    # already SP x true scale (x*32 times A*1024), which matches the main
    # psum scale, so the gate is just the per-slot scaling.
    slot = np.asarray(token_to_slot).reshape(NCORES, TC)
    g = (slot[:, None, :] == np.arange(S, dtype=slot.dtype)[None, :, None])
    g = g.astype(f) * np.asarray(scaling, dtype=f)[None, :, None]
    gate = np.ascontiguousarray(np.repeat(g, R, axis=1))

    in_maps = []
    for c in range(NCORES):
        in_maps.append({
            "x_sh": x_sh[c],
            "w_t": w_t,
            "w8_t": w8,
            "pax": pax[c],
            "b_t": b_t,
            "gate": gate[c],
        })
    return in_maps


def _assemble(results):
    out = np.empty((T, D), dtype=np.float32)
    for c in range(NCORES):
        out[c * TC:(c + 1) * TC, :] = results[c]["y_out"].reshape(D, TC).T
    return out


def _run(inputs, trace=False):
    from concourse.bass_utils import run_bass_kernel_spmd
    nc = _get_nc()
    in_maps = _prep_in_maps(**inputs)
    res = run_bass_kernel_spmd(
        nc, in_maps, core_ids=list(range(NCORES)), trace=trace)
    return res


def kernel(**inputs) -> np.ndarray:
    res = _run(inputs, trace=False)
    return _assemble(res.results)


if __name__ == "__main__":
    rng = np.random.default_rng(0)
    ins = {
        "x": rng.standard_normal((T, HID)).astype(np.float32),
        "W": (rng.standard_normal((D, HID)) * 0.02).astype(np.float32),
        "lora_A": (rng.standard_normal((S, 3, R, HID)) * 0.02).astype(np.float32),
        "lora_B_q": (rng.standard_normal((S, Q_SIZE, R)) * 0.02).astype(np.float32),
        "lora_B_k": (rng.standard_normal((S, KV_SIZE, R)) * 0.02).astype(np.float32),
        "lora_B_v": (rng.standard_normal((S, KV_SIZE, R)) * 0.02).astype(np.float32),
        "scaling": rng.uniform(0.5, 2.0, S).astype(np.float32),
        "token_to_slot": rng.integers(0, S, T).astype(np.int32),
    }
    out = kernel(**ins)
    print("out", out.shape, out.dtype)



# revision 2
# speedup vs baseline: 1.0357x; 1.0357x over previous
"""LoRA QKV fused projection kernel for 8 TRN2 NeuronCores.

Reference computation (T=8192 tokens, HID=4096, D=6144 out, S=8 slots, R=16):
    y = x @ W.T
    a[t,s,i,r] = sum_h x[t,h] * lora_A[s,i,r,h]         (down-proj, all slots)
    a *= onehot(token_to_slot)[t,s] * scaling[s]         (routing gate)
    d[t, :] = concat_i( sum_{s,r} a[t,s,i,r] * B_i[s,:,r] )   (up-proj)
    out = y + d
Sharding: data-parallel over tokens; core c owns tokens [c*1024, (c+1)*1024).

Per-core dataflow (v2):
  * Phase A (LoRA down-proj aT = A @ x) in fp8 e4m3 DoubleRow over all 16
    k-tile PAIRS, j-major into 6 psum chains (3 targets x 2 token halves).
    The packed pax table ([A targets | x tokens] per (j,pair) row) streams
    in j-chunks on the scalar queue, pacing the A matmuls.
  * mb0's main k-chain is interleaved BETWEEN Phase A j-groups: its bf16
    k-tiles are paced by the x_res chunks arriving on the sync queue, so
    the PE stays fed from ~8us while both streams warm up (banks: 6 A +
    2 for mb0 = 8 exactly; mb0's B-matmul lands right after the gate).
  * Phase B: routing gate (host-built onehot*scaling, expanded over rank)
    applied on DVE: ag = psum_a * gate, written bf16.
  * Phase C per output row-block mb (48): 24 bf16 k-tiles (W pre-scaled by
    SP=2^15 to match the fp8 psum scale) + 4 fp8 DoubleRow k-tile PAIRS
    (reusing the pax x8 slices) accumulate into 2 psum tiles (token
    halves); the LoRA up-proj B[mb] @ ag[i] accumulates INTO THE SAME
    psum (start=False), fusing base + delta. psum->sbuf copy descales by
    1/SP exactly, then one DMA out per half (last 2 mb split into 256-col
    pieces to shorten the tail).

Precision (gate 2e-2): bf16 main + fp8 on 8/32 k-tiles + fp8 LoRA delta
measured 1.921e-2 on the harness inputs (numpy sim matches HW to ~1e-6).
fp8 e4m3 DoubleRow measured ~234ns per 512-col pair vs 2x218ns bf16.
"""

import numpy as np
import ml_dtypes

# problem shape (hardcoded per harness contract)
T = 8192
HID = 4096
Q_SIZE = 4096
KV_SIZE = 1024
D = Q_SIZE + 2 * KV_SIZE  # 6144
S = 8
R = 16
NCORES = 8
P = 128

TC = T // NCORES          # 1024 tokens per core
MB = D // P               # 48 output row-blocks of 128
KA = HID // P             # 32 k-tiles
NP8 = 4                   # k-tile PAIRS of the main GEMM done in fp8 DoubleRow
KB = KA - 2 * NP8         # 24 k-tiles of the main GEMM done in bf16
NH = TC // 512            # 2 token halves (psum bank = 512 fp32)
JA = KA // 2              # 16 k-tile pairs
SP = np.float32(32768.0)  # psum scale SX*SA: bf16 W is pre-scaled by SP so
                          # fp8 (x*32)(W*1024) products accumulate coherently;
                          # the final psum->sbuf copy descales by 1/SP (exact)

_CACHE = {}


def _build_nc():
    import concourse.mybir as mybir
    import concourse.tile as tile
    from concourse import bacc

    bf16 = mybir.dt.bfloat16
    f32 = mybir.dt.float32
    f8 = mybir.dt.float8e4
    DR = mybir.MatmulPerfMode.DoubleRow

    nc = bacc.Bacc(None, target_bir_lowering=False, debug=False)

    # ---- DRAM parameters (per-core shapes)
    PAX = 3 * P + TC  # per-(j,pair) packed row: 3 A targets then x tokens
    x_d = nc.declare_dram_parameter("x_sh", [P, KB, TC], bf16, isOutput=False)
    w_d = nc.declare_dram_parameter("w_t", [MB, P, KB, P], bf16, isOutput=False)
    w8_d = nc.declare_dram_parameter("w8_t", [MB, P, NP8, 2, P], f8, isOutput=False)
    pax_d = nc.declare_dram_parameter("pax", [P, JA, 2, PAX], f8, isOutput=False)
    b_d = nc.declare_dram_parameter("b_t", [P, MB, P], bf16, isOutput=False)
    g_d = nc.declare_dram_parameter("gate", [P, TC], f32, isOutput=False)
    y_d = nc.declare_dram_parameter("y_out", [MB, P, TC], f32, isOutput=True)

    with tile.TileContext(nc) as tc:
        with tc.tile_pool(name="xres", bufs=1) as xres_pool, \
             tc.tile_pool(name="wp", bufs=3) as w_pool, \
             tc.tile_pool(name="ab", bufs=1) as ab_pool, \
             tc.tile_pool(name="agp", bufs=1) as ag_pool, \
             tc.tile_pool(name="stp", bufs=3) as st_pool, \
             tc.tile_pool(name="psum", bufs=8, space="PSUM") as ps_pool:

            # resident operands
            x_res = xres_pool.tile([P, KB, TC], bf16, tag="xres")
            pax_t = ab_pool.tile([P, JA, 2, PAX], f8, tag="pax")
            b_t = ab_pool.tile([P, MB, P], bf16, tag="b")
            gate_t = ab_pool.tile([P, TC], f32, tag="gate")

            # ---- sync queue: gate first (needed at ~40us), then x chunks
            # pacing mb0's k-chain, early b slice, rest of x, rest of b.
            nc.sync.dma_start(out=gate_t[:], in_=g_d[:])
            XCH = [4, 4, 4, 4, 4, 4]  # x_res k-tile chunks
            xc = []
            c0 = 0
            for cw in XCH:
                xc.append((c0, min(cw, KB - c0)))
                c0 += cw
            nc.sync.dma_start(out=x_res[:, xc[0][0]:xc[0][0] + xc[0][1], :],
                              in_=x_d[:, xc[0][0]:xc[0][0] + xc[0][1], :])
            nc.sync.dma_start(out=b_t[:, 0:8], in_=b_d[:, 0:8])
            for (ch0, cw) in xc[1:]:
                nc.sync.dma_start(out=x_res[:, ch0:ch0 + cw, :],
                                  in_=x_d[:, ch0:ch0 + cw, :])
            nc.sync.dma_start(out=b_t[:, 8:MB], in_=b_d[:, 8:MB])

            # ---- scalar queue: w0 first, then pax j-chunks (pacing Phase A),
            # then the rest of the W stream.
            def load_w(mb):
                w_t = w_pool.tile([P, KB, P], bf16, tag="w", name=f"w{mb}")
                nc.scalar.dma_start(out=w_t[:], in_=w_d[mb])
                w8_t = w_pool.tile([P, NP8, 2, P], f8, tag="w8", name=f"w8{mb}")
                nc.scalar.dma_start(out=w8_t[:], in_=w8_d[mb])
                return w_t, w8_t

            w0_t, w80_t = load_w(0)

            jch = [2, 2, 4, 4, 4]  # pax j-tiles per streamed chunk
            j0 = 0
            for jc in jch:
                nc.scalar.dma_start(
                    out=pax_t[:, j0:j0 + jc], in_=pax_d[:, j0:j0 + jc])
                j0 += jc

            # ---------------- PE emission ------------------------------------
            # Phase A psums (6 banks) + mb0's 2 psums = 8 banks.
            ps_a = [
                ps_pool.tile([P, 512], f32, tag="ps", name=f"ps_a{i}_{h}")
                for i in range(3) for h in range(NH)
            ]
            ps0 = [ps_pool.tile([P, 512], f32, tag="ps", name=f"pm0_{h}")
                   for h in range(NH)]

            def a_jgroup(jlo, jhi):
                for j in range(jlo, jhi):
                    for i in range(3):
                        for h in range(NH):
                            nc.tensor.matmul(
                                ps_a[i * NH + h][:],
                                pax_t[:, j, :, i * P:(i + 1) * P],
                                pax_t[:, j, :, 3 * P + h * 512:3 * P + (h + 1) * 512],
                                start=(j == 0), stop=(j == JA - 1),
                                perf_mode=DR,
                            )

            def mb0_ksub(klo, khi):
                for k in range(klo, khi):
                    for h in range(NH):
                        nc.tensor.matmul(
                            ps0[h][:],
                            w0_t[:, k, :],
                            x_res[:, k, h * 512:(h + 1) * 512],
                            start=(k == 0), stop=False,
                        )

            # Interleave: A j-groups (paced by pax on scalar) with mb0
            # k-subchains (paced by x_res on sync).
            a_jgroup(0, 2)
            mb0_ksub(0, 4)
            a_jgroup(2, 4)
            mb0_ksub(4, 8)
            a_jgroup(4, 8)
            mb0_ksub(8, 12)
            a_jgroup(8, 12)
            mb0_ksub(12, 16)
            a_jgroup(12, JA)
            mb0_ksub(16, 20)
            # mb0 fp8 pairs (pax j=12..15 now resident)
            for jp in range(NP8):
                j = KB // 2 + jp
                for h in range(NH):
                    nc.tensor.matmul(
                        ps0[h][:],
                        w80_t[:, jp, :, :],
                        pax_t[:, j, :, 3 * P + h * 512:3 * P + (h + 1) * 512],
                        start=False, stop=False,
                        perf_mode=DR,
                    )
            mb0_ksub(20, KB)

            # ---------------- Phase B: routing gate ------------------------
            ag = []
            for i in range(3):
                ag_t = ag_pool.tile([P, TC], bf16, tag=f"ag{i}", name=f"ag{i}")
                for h in range(NH):
                    sl = slice(h * 512, (h + 1) * 512)
                    nc.vector.tensor_mul(ag_t[:, sl], ps_a[i * NH + h][:], gate_t[:, sl])
                ag.append(ag_t)

            # ---------------- mb0 B-matmul + copy-out ----------------------
            def finish_mb(mb, pss):
                i = 0 if mb < Q_SIZE // P else (1 if mb < (Q_SIZE + KV_SIZE) // P else 2)
                st = st_pool.tile([P, TC], f32, tag="st", name=f"st{mb}")
                # split the very last output DMAs to shorten the tail
                pieces = 2 if mb >= MB - 2 else 1
                for h in range(NH):
                    nc.tensor.matmul(
                        pss[h][:],
                        b_t[:, mb, :],
                        ag[i][:, h * 512:(h + 1) * 512],
                        start=False, stop=True,
                    )
                    nc.vector.tensor_scalar_mul(
                        st[:, h * 512:(h + 1) * 512], pss[h][:],
                        float(1.0 / SP))
                    pw = 512 // pieces
                    for pc in range(pieces):
                        lo = h * 512 + pc * pw
                        nc.sync.dma_start(
                            out=y_d[mb, :, lo:lo + pw],
                            in_=st[:, lo:lo + pw],
                        )

            finish_mb(0, ps0)

            # ------------- Phase C: remaining mb chains ---------------------
            for mb in range(1, MB):
                w_t, w8_t = load_w(mb)
                pss = [
                    ps_pool.tile([P, 512], f32, tag="ps", name=f"pm{mb}_{h}")
                    for h in range(NH)
                ]
                for k in range(KB):
                    for h in range(NH):
                        nc.tensor.matmul(
                            pss[h][:],
                            w_t[:, k, :],
                            x_res[:, k, h * 512:(h + 1) * 512],
                            start=(k == 0), stop=False,
                        )
                for jp in range(NP8):
                    j = KB // 2 + jp
                    for h in range(NH):
                        nc.tensor.matmul(
                            pss[h][:],
                            w8_t[:, jp, :, :],
                            pax_t[:, j, :, 3 * P + h * 512:3 * P + (h + 1) * 512],
                            start=False, stop=False,
                            perf_mode=DR,
                        )
                finish_mb(mb, pss)

    nc.compile()
    return nc


def _get_nc():
    if "nc" not in _CACHE:
        _CACHE["nc"] = _build_nc()
    return _CACHE["nc"]


def _prep_in_maps(x, W, lora_A, lora_B_q, lora_B_k, lora_B_v, scaling, token_to_slot):
    f = np.float32
    bf = ml_dtypes.bfloat16
    x = np.ascontiguousarray(x, dtype=f)
    W = np.ascontiguousarray(W, dtype=f)

    # x shard, moving operand: [c, p, ka, tl]  (h = ka*128 + p, t = c*1024 + tl)
    x_f32 = np.ascontiguousarray(
        x.reshape(NCORES, TC, KA, P).transpose(0, 3, 2, 1))
    x_sh = np.ascontiguousarray(x_f32[:, :, :KB, :]).astype(bf)
    # W stationary: [mb, p, ka, dl]  (d = mb*128 + dl)  -- replicated.
    # bf16 part pre-scaled by SP to match the fp8 psum scale; the last
    # 2*NP8 k-tiles go as fp8(W*1024) DoubleRow pairs.
    w_all = W.reshape(MB, P, KA, P).transpose(0, 3, 2, 1)
    w_t = np.ascontiguousarray(w_all[:, :, :KB, :] * SP).astype(bf)
    # fp8 e4m3 copies for the LoRA down-proj (DoubleRow pairs of k-tiles),
    # packed [A targets | x tokens] per (j, pair) row so each chunk is one
    # DMA; the 1/(SX*SA) descale folds into the gate below.
    SX, SA = np.float32(32.0), np.float32(1024.0)
    f8 = ml_dtypes.float8_e4m3
    w8 = np.ascontiguousarray(
        (w_all[:, :, KB:, :] * SA).astype(f8).reshape(MB, P, NP8, 2, P))
    x8 = (x_f32 * SX).astype(f8).reshape(NCORES, P, JA, 2, TC)
    a_f32 = np.ascontiguousarray(
        np.asarray(lora_A, dtype=f).reshape(S, 3, R, KA, P).transpose(4, 3, 1, 0, 2)
        .reshape(P, KA, 3, S * R))
    a8 = (a_f32 * SA).astype(f8).reshape(P, JA, 2, 3 * S * R)
    pax = np.concatenate(
        [np.broadcast_to(a8, (NCORES,) + a8.shape), x8], axis=-1)
    pax = np.ascontiguousarray(pax)
    # LoRA B stationary: [(s r), mb, dl] -- replicated
    bq = np.asarray(lora_B_q, dtype=f).transpose(0, 2, 1).reshape(S * R, Q_SIZE)
    bk = np.asarray(lora_B_k, dtype=f).transpose(0, 2, 1).reshape(S * R, KV_SIZE)
    bv = np.asarray(lora_B_v, dtype=f).transpose(0, 2, 1).reshape(S * R, KV_SIZE)
    b_t = np.ascontiguousarray(
        np.concatenate([bq, bk, bv], axis=1).reshape(S * R, MB, P)).astype(bf)
    # routing gate, expanded over ranks: [c, (s r), tl]. The LoRA psum is
    # already SP x true scale (x*32 times A*1024), which matches the main
    # psum scale, so the gate is just the per-slot scaling.
    slot = np.asarray(token_to_slot).reshape(NCORES, TC)
    g = (slot[:, None, :] == np.arange(S, dtype=slot.dtype)[None, :, None])
    g = g.astype(f) * np.asarray(scaling, dtype=f)[None, :, None]
    gate = np.ascontiguousarray(np.repeat(g, R, axis=1))

    in_maps = []
    for c in range(NCORES):
        in_maps.append({
            "x_sh": x_sh[c],
            "w_t": w_t,
            "w8_t": w8,
            "pax": pax[c],
            "b_t": b_t,
            "gate": gate[c],
        })
    return in_maps


def _assemble(results):
    out = np.empty((T, D), dtype=np.float32)
    for c in range(NCORES):
        out[c * TC:(c + 1) * TC, :] = results[c]["y_out"].reshape(D, TC).T
    return out


def _run(inputs, trace=False):
    from concourse.bass_utils import run_bass_kernel_spmd
    nc = _get_nc()
    in_maps = _prep_in_maps(**inputs)
    res = run_bass_kernel_spmd(
        nc, in_maps, core_ids=list(range(NCORES)), trace=trace)
    return res


def kernel(**inputs) -> np.ndarray:
    res = _run(inputs, trace=False)
    return _assemble(res.results)


if __name__ == "__main__":
    rng = np.random.default_rng(0)
    ins = {
        "x": rng.standard_normal((T, HID)).astype(np.float32),
        "W": (rng.standard_normal((D, HID)) * 0.02).astype(np.float32),
        "lora_A": (rng.standard_normal((S, 3, R, HID)) * 0.02).astype(np.float32),
        "lora_B_q": (rng.standard_normal((S, Q_SIZE, R)) * 0.02).astype(np.float32),
        "lora_B_k": (rng.standard_normal((S, KV_SIZE, R)) * 0.02).astype(np.float32),
        "lora_B_v": (rng.standard_normal((S, KV_SIZE, R)) * 0.02).astype(np.float32),
        "scaling": rng.uniform(0.5, 2.0, S).astype(np.float32),
        "token_to_slot": rng.integers(0, S, T).astype(np.int32),
    }
    out = kernel(**ins)
    print("out", out.shape, out.dtype)
